# revision 1
# baseline (speedup 1.0000x reference)
"""GCN-style message passing kernel for Trainium2 (8 NeuronCores).

Math (see reference):
    deg    = diag(D)                      (== row sums of A by construction)
    j0(i)  = argmax_j (A[i,j] > 0)        (first neighbor; self-loops ensure >=1)
    coeff  = A * outer(1/sqrt(deg[j0]), 1/sqrt(deg))
    out    = leaky_relu((coeff @ X) @ W.T + b, 0.01)

Decomposition per core (rows sharded, 1024 rows/core):
    agg   = diag(r0) @ A_sh @ (diag(r) @ X)       r = 1/sqrt(deg), r0 = 1/sqrt(deg[j0])
    out   = leaky_relu(agg @ W.T + b)

A is 0/1 so it is exact in bf16. The big product A_sh @ Xs runs on the
TensorEngine with A^T tiles as the stationary operand (A^T obtained via
hardware DMA-transpose on load). deg[j0] is recovered on-device:
  - a side matmul with a "position" matrix W2 (w[j] = 2^(-2*(j%64)), one
    column per 64-node chunk) produces s[i,c] whose f32 EXPONENT encodes the
    first neighbor's offset within chunk c (sum of distinct 2-bit-spaced
    powers of two can never carry into the next exponent slot),
  - bit tricks + a free-dim min-reduce give first_j = 64*c* + jl*,
  - deg[first_j] is then gathered with a tiny bilinear form:
    onehot(c*)^T @ Dmat dotted with onehot(jl*), Dmat[q,r] = deg[64q+r].
"""

import numpy as np
import ml_dtypes

BF16 = ml_dtypes.bfloat16

N_NODES = 8192
F_IN = 256
F_OUT = 256
N_CORES = 8
ROWS = N_NODES // N_CORES  # rows per core

# accuracy mode: 'exact' = f32 split into two bf16 passes (err ~1e-5),
# 'fp16' = single fp16 pass (err ~5e-4), 'bf16' = single bf16 pass (~3e-3).
EXACT = 'exact'

_BUILT = {}


def _build_nc(rows, n_nodes, f_in, f_out, mode, debug=False, repeat=1, stage=99):
    exact = (mode == 'exact') or (mode is True)
    import concourse.bass as bass
    import concourse.tile as tile
    from concourse import bacc, mybir

    f32 = mybir.dt.float32
    bf = mybir.dt.float16 if mode == 'fp16' else mybir.dt.bfloat16
    i32 = mybir.dt.int32
    u32 = mybir.dt.uint32
    Alu = mybir.AluOpType

    n_jblk = n_nodes // 128     # contraction blocks
    n_iblk = rows // 128        # output row blocks per core
    C = n_nodes // 128          # 128-node chunks (s columns) == n_jblk
    NB = n_jblk
    assert C <= 128 and n_nodes % 128 == 0 and rows % 128 == 0
    assert f_in % 128 == 0 and f_out <= 512

    nc = bacc.Bacc("TRN2", target_bir_lowering=False, debug=False)
    a_sh = nc.dram_tensor("a_sh", [rows, n_nodes], bf, kind="ExternalInput")
    dvec = nc.dram_tensor("dvec", [n_nodes], f32, kind="ExternalInput")
    if exact:
        x_in = nc.dram_tensor("x_f32", [n_nodes, f_in], f32, kind="ExternalInput")
    else:
        x_in = nc.dram_tensor("x_bf", [n_nodes, f_in], bf, kind="ExternalInput")
    w_t = nc.dram_tensor("w_t", [f_in, f_out], f32, kind="ExternalInput")
    bias_row = nc.dram_tensor("bias_row", [128, f_out], f32, kind="ExternalInput")
    w2reg_d = nc.dram_tensor("w2reg", [128, n_jblk, C], bf, kind="ExternalInput")
    ident_d = nc.dram_tensor("ident", [128, 128], bf, kind="ExternalInput")
    i2c227_d = nc.dram_tensor("i2c227", [128, C], i32, kind="ExternalInput")
    iq_d = nc.dram_tensor("iota_q", [128, C], f32, kind="ExternalInput")
    ir_d = nc.dram_tensor("iota_r", [128, 128], f32, kind="ExternalInput")
    out_d = nc.dram_tensor("out_sh", [rows, f_out], f32, kind="ExternalOutput")
    if debug:
        dbg_s = nc.dram_tensor("dbg_s", [rows, C], f32, kind="ExternalOutput")
        dbg_kmin = nc.dram_tensor("dbg_kmin", [rows, 1], i32, kind="ExternalOutput")
        dbg_dj0 = nc.dram_tensor("dbg_dj0", [rows, 1], f32, kind="ExternalOutput")
        dbg_agg = nc.dram_tensor("dbg_agg", [rows, f_in], f32, kind="ExternalOutput")
        dbg_at = nc.dram_tensor("dbg_at", [128, rows], f32, kind="ExternalOutput")
        dbg_xs = nc.dram_tensor("dbg_xs", [128, f_in], f32, kind="ExternalOutput")

    nfi = f_in // 128  # fi blocks for second matmul

    with tile.TileContext(nc) as tc:
        with (
            tc.tile_pool(name="singles", bufs=1) as singles,
            tc.tile_pool(name="apool", bufs=4) as apool,
            tc.tile_pool(name="xpool", bufs=3) as xpool,
            tc.tile_pool(name="work", bufs=2) as work,
            tc.tile_pool(name="pspool", bufs=8, space="PSUM") as pspool,
        ):
            # ---- constants / prep ----
            ident = singles.tile([128, 128], bf)
            nc.gpsimd.dma_start(ident[:], ident_d[:])
            i2c227 = singles.tile([128, C], i32)
            nc.gpsimd.dma_start(i2c227[:], i2c227_d[:])
            iq = singles.tile([128, C], f32)
            nc.gpsimd.dma_start(iq[:], iq_d[:])
            ir = singles.tile([128, 128], f32)
            nc.gpsimd.dma_start(ir[:], ir_d[:])
            bias_t = singles.tile([128, f_out], f32)
            nc.gpsimd.dma_start(bias_t[:], bias_row[:])

            # degrees: r = 1/sqrt(deg) laid out [p, nb]; Dmat[q, r] = deg[64q+r]
            dvec_t = singles.tile([128, NB], f32)
            nc.gpsimd.dma_start(dvec_t[:], dvec[:].rearrange("(nb p) -> p nb", p=128))
            dmat_f = singles.tile([C, 128], f32)
            nc.gpsimd.dma_start(dmat_f[:], dvec[:].rearrange("(q r) -> q r", r=128))
            dmat_b = singles.tile([C, 128], bf)
            nc.vector.tensor_copy(dmat_b[:], dmat_f[:])

            sq_t = singles.tile([128, NB], f32)
            nc.scalar.sqrt(sq_t[:], dvec_t[:])
            r_t = singles.tile([128, NB], f32)
            nc.vector.reciprocal(r_t[:], sq_t[:])

            # W^T in bf16 hi/lo: wthi/wtlo [128, nfi, f_out]
            wt_f = singles.tile([128, nfi, f_out], f32)
            nc.gpsimd.dma_start(
                wt_f[:], w_t[:].rearrange("(nf p) fo -> p nf fo", p=128)
            )
            wthi = singles.tile([128, nfi, f_out], bf)
            nc.vector.tensor_copy(wthi[:], wt_f[:])
            wtlo = singles.tile([128, nfi, f_out], bf)
            nc.vector.tensor_sub(wtlo[:], wt_f[:], wthi[:])

            assert repeat == 1 or not debug
            for _rep in range(repeat):
                # ---- moving operand: [Xs | W2] per j-block (bf16), + lo if exact
                xsw = singles.tile([128, n_jblk, f_in + C], bf)
                for jb in range(n_jblk):
                    nc.gpsimd.dma_start(
                        xsw[:, jb, f_in:f_in + C], w2reg_d[:, jb, :]
                    )
                if exact:
                    xs_lo = singles.tile([128, n_jblk, f_in], bf)
                for jb in range(n_jblk):
                    if exact:
                        xst = xpool.tile([128, f_in], f32, tag="xst")
                        nc.sync.dma_start(xst[:], x_in[jb * 128:(jb + 1) * 128, :])
                        xsf = xpool.tile([128, f_in], f32, tag="xsf")
                        nc.vector.tensor_scalar_mul(xsf[:], xst[:], r_t[:, jb:jb + 1])
                        nc.vector.tensor_copy(xsw[:, jb, 0:f_in], xsf[:])
                        nc.vector.tensor_sub(xs_lo[:, jb, :], xsf[:], xsw[:, jb, 0:f_in])
                    else:
                        xst = xpool.tile([128, f_in], bf, tag="xst")
                        nc.gpsimd.dma_start(xst[:], x_in[jb * 128:(jb + 1) * 128, :])
                        nc.vector.tensor_scalar_mul(
                            xsw[:, jb, 0:f_in], xst[:], r_t[:, jb:jb + 1]
                        )

                if stage <= 1:
                    for ib in range(n_iblk):
                        zz = work.tile([128, f_out], f32, tag="zz")
                        nc.vector.tensor_copy(zz[:], xsw[:, ib, 0:f_out])
                        nc.sync.dma_start(out_d[ib * 128:(ib + 1) * 128, :], zz[:])
                    continue
                # ---- main accumulation: agg = A_sh @ Xs ; s = A_sh @ W2
                ps_main = [
                    pspool.tile([128, f_in + C], f32, tag="ps", name=f"ps_main{i}")
                    for i in range(n_iblk)
                ]
                for jb in range(n_jblk):
                    aslab = apool.tile([128, rows], bf, tag="aslab")
                    nc.sync.dma_start(
                        aslab[:], a_sh[:, jb * 128:(jb + 1) * 128], transpose=True
                    )
                    if debug and jb == 0:
                        a_dump = work.tile([128, rows], f32, tag="a_dump")
                        nc.vector.tensor_copy(a_dump[:], aslab[:])
                        nc.sync.dma_start(dbg_at[:], a_dump[:])
                        x_dump = work.tile([128, f_in], f32, tag="x_dump")
                        nc.vector.tensor_copy(x_dump[:], xsw[:, jb, 0:f_in])
                        nc.sync.dma_start(dbg_xs[:], x_dump[:])
                    for ib in range(n_iblk):
                        lhsT = aslab[:, ib * 128:(ib + 1) * 128]
                        nc.tensor.matmul(
                            ps_main[ib][:, 0:f_in + C],
                            lhsT,
                            xsw[:, jb, :],
                            start=(jb == 0),
                            stop=(jb == n_jblk - 1) and not exact,
                        )
                        if exact:
                            nc.tensor.matmul(
                                ps_main[ib][:, 0:f_in],
                                lhsT,
                                xs_lo[:, jb, :],
                                start=False,
                                stop=(jb == n_jblk - 1),
                            )

                if stage <= 2:
                    for ib in range(n_iblk):
                        agg_raw = work.tile([128, f_in], f32, tag="agg_raw")
                        nc.scalar.copy(agg_raw[:], ps_main[ib][:, 0:f_in])
                        nc.sync.dma_start(out_d[ib * 128:(ib + 1) * 128, :], agg_raw[:])
                    continue
                # ---- per row-block epilogue ----
                for ib in range(n_iblk):
                    # drain psum: s and unscaled agg -> SBUF (releases the bank)
                    s_sb = work.tile([128, C], f32, tag="s_sb")
                    nc.scalar.copy(s_sb[:], ps_main[ib][:, f_in:f_in + C])
                    agg_raw = work.tile([128, f_in], f32, tag="agg_raw")
                    nc.scalar.copy(agg_raw[:], ps_main[ib][:, 0:f_in])
                    if stage <= 30:
                        continue
                    e_u = work.tile([128, C], i32, tag="e_u")
                    nc.vector.tensor_scalar(
                        e_u[:], s_sb[:].bitcast(i32), 23, None,
                        op0=Alu.logical_shift_right,
                    )
                    key = work.tile([128, C], i32, tag="key")
                    nc.vector.scalar_tensor_tensor(
                        key[:], e_u[:], -1, i2c227[:], op0=Alu.mult, op1=Alu.add
                    )
                    msk = work.tile([128, C], i32, tag="msk")
                    nc.vector.tensor_scalar(
                        msk[:], e_u[:], 0, 1 << 20, op0=Alu.is_equal, op1=Alu.mult
                    )
                    key2 = work.tile([128, C], i32, tag="key2")
                    nc.vector.tensor_tensor(key2[:], key[:], msk[:], Alu.add)
                    kmin = work.tile([128, 1], i32, tag="kmin")
                    nc.vector.tensor_reduce(
                        kmin[:], key2[:], axis=mybir.AxisListType.X, op=Alu.min
                    )
                    # kmin = 256*c + jl  (c = chunk, jl = offset in chunk)
                    jl2_i = work.tile([128, 1], i32, tag="jl2_i")
                    nc.vector.tensor_scalar(
                        jl2_i[:], kmin[:], 127, None, op0=Alu.bitwise_and
                    )
                    c128_i = work.tile([128, 1], i32, tag="c128_i")
                    nc.vector.tensor_scalar(
                        c128_i[:], kmin[:], -256, None, op0=Alu.bitwise_and
                    )
                    if stage <= 31:
                        continue
                    jl2_f = work.tile([128, 1], f32, tag="jl2_f")
                    nc.vector.tensor_copy(jl2_f[:], jl2_i[:])
                    c128_f = work.tile([128, 1], f32, tag="c128_f")
                    nc.vector.tensor_copy(c128_f[:], c128_i[:])

                    if stage <= 32:
                        continue
                    # onehots; gather deg[first_j] via oq^T @ Dmat then dot with or
                    oq = work.tile([128, C], bf, tag="oq")
                    nc.vector.tensor_scalar(
                        oq[:], iq[:], c128_f[:], None, op0=Alu.is_equal
                    )
                    orf = work.tile([128, 128], f32, tag="orf")
                    nc.vector.tensor_scalar(
                        orf[:], ir[:], jl2_f[:], None, op0=Alu.is_equal
                    )
                    if stage <= 33:
                        continue
                    p_oqT = pspool.tile([C, 128], bf, tag="ps")
                    nc.tensor.transpose(p_oqT[:], oq[:], ident[:])
                    oqT = work.tile([C, 128], bf, tag="oqT")
                    nc.scalar.copy(oqT[:], p_oqT[:])
                    if stage <= 34:
                        continue
                    t1 = pspool.tile([128, 128], f32, tag="ps")
                    nc.tensor.matmul(t1[:], oqT[:], dmat_b[:], start=True, stop=True)
                    if stage <= 35:
                        continue
                    t1s = work.tile([128, 128], f32, tag="t1s")
                    nc.scalar.copy(t1s[:], t1[:])
                    ttr_scr = work.tile([128, 128], f32, tag="ttr_scr")
                    nc.vector.tensor_tensor(ttr_scr[:], t1s[:], orf[:], Alu.mult)
                    dj0 = work.tile([128, 1], f32, tag="dj0")
                    nc.vector.reduce_sum(
                        dj0[:], ttr_scr[:], axis=mybir.AxisListType.X
                    )
                    if debug:
                        nc.sync.dma_start(dbg_s[ib * 128:(ib + 1) * 128, :], s_sb[:])
                        nc.sync.dma_start(dbg_kmin[ib * 128:(ib + 1) * 128, :], kmin[:])
                        nc.sync.dma_start(dbg_dj0[ib * 128:(ib + 1) * 128, :], dj0[:])
                    if stage <= 3:
                        nc.sync.dma_start(
                            out_d[ib * 128:(ib + 1) * 128, 0:C], s_sb[:]
                        )
                        continue
                    sq0 = work.tile([128, 1], f32, tag="sq0")
                    nc.scalar.sqrt(sq0[:], dj0[:])
                    r0 = work.tile([128, 1], f32, tag="r0")
                    nc.vector.reciprocal(r0[:], sq0[:])

                    # agg scaled by r0, cast, transpose for the W matmul
                    if exact:
                        agg_f = work.tile([128, f_in], f32, tag="agg_f")
                        nc.vector.tensor_scalar_mul(agg_f[:], agg_raw[:], r0[:])
                        agg_b = work.tile([128, f_in], bf, tag="agg_b")
                        nc.vector.tensor_copy(agg_b[:], agg_f[:])
                        agg_l = work.tile([128, f_in], bf, tag="agg_l")
                        nc.vector.tensor_sub(agg_l[:], agg_f[:], agg_b[:])
                    else:
                        agg_b = work.tile([128, f_in], bf, tag="agg_b")
                        nc.vector.tensor_scalar_mul(agg_b[:], agg_raw[:], r0[:])
                    if debug:
                        agg_dump = work.tile([128, f_in], f32, tag="agg_dump")
                        nc.vector.tensor_copy(agg_dump[:], agg_b[:])
                        nc.sync.dma_start(dbg_agg[ib * 128:(ib + 1) * 128, :], agg_dump[:])

                    aggTs = []
                    for h in range(nfi):
                        p_aT = pspool.tile([128, 128], bf, tag="ps")
                        nc.tensor.transpose(
                            p_aT[:], agg_b[:, h * 128:(h + 1) * 128], ident[:]
                        )
                        aT = work.tile([128, 128], bf, tag=f"aT{h}")
                        nc.scalar.copy(aT[:], p_aT[:])
                        aggTs.append(aT)
                    if exact:
                        aggTls = []
                        for h in range(nfi):
                            p_aT = pspool.tile([128, 128], bf, tag="ps")
                            nc.tensor.transpose(
                                p_aT[:], agg_l[:, h * 128:(h + 1) * 128], ident[:]
                            )
                            aTl = work.tile([128, 128], bf, tag=f"aTl{h}")
                            nc.scalar.copy(aTl[:], p_aT[:])
                            aggTls.append(aTl)

                    ps2 = pspool.tile([128, f_out], f32, tag="ps")
                    prods = []
                    for h in range(nfi):
                        prods.append((aggTs[h], wthi[:, h, :]))
                        prods.append((aggTs[h], wtlo[:, h, :]))
                        if exact:
                            prods.append((aggTls[h], wthi[:, h, :]))
                    for pi, (lhs, rhs) in enumerate(prods):
                        nc.tensor.matmul(
                            ps2[:], lhs[:], rhs,
                            start=(pi == 0), stop=(pi == len(prods) - 1),
                        )

                    z = work.tile([128, f_out], f32, tag="z")
                    nc.vector.tensor_add(z[:], ps2[:], bias_t[:])
                    out_t = work.tile([128, f_out], f32, tag="out_t")
                    nc.vector.scalar_tensor_tensor(
                        out_t[:], z[:], 0.01, z[:], op0=Alu.mult, op1=Alu.max
                    )
                    nc.sync.dma_start(
                        out_d[ib * 128:(ib + 1) * 128, :], out_t[:]
                    )

    nc.finalize()
    return nc


def _get_nc(rows, n_nodes, f_in, f_out, mode, debug=False, repeat=1, stage=99):
    key = (rows, n_nodes, f_in, f_out, mode, debug, repeat, stage)
    if key not in _BUILT:
        _BUILT[key] = _build_nc(*key)
    return _BUILT[key]


def host_inputs(D, X, A, W, b, n_cores=N_CORES, mode=EXACT):
    """Build per-core input maps (pure slicing / dtype re-encoding)."""
    exact = (mode == 'exact') or (mode is True)
    FDT = np.float16 if mode == 'fp16' else BF16
    n, f_in = X.shape
    f_out = W.shape[0]
    rows = n // n_cores
    C = n // 128
    nb = n // 128

    # A is 0/1: cast to 16-bit float is exact
    if mode == 'fp16':
        A_bf = np.ascontiguousarray(A).astype(np.float16).view(np.uint16)
    else:
        A_bf = (np.ascontiguousarray(A).view(np.uint32) >> 16).astype(np.uint16)
    dvec = np.ascontiguousarray(np.diagonal(D)).astype(np.float32)
    w_t = np.ascontiguousarray(W.T).astype(np.float32)
    bias_row = np.broadcast_to(b.astype(np.float32), (128, f_out)).copy()

    n_jblk = n // 128
    p = np.arange(128)
    w2reg = np.zeros((128, n_jblk, C), dtype=FDT)
    vals = (2.0 ** (100.0 - p)).astype(FDT)
    for bb in range(n_jblk):
        w2reg[p, bb, bb] = vals

    ident = np.eye(128, dtype=FDT)
    i2c227 = np.broadcast_to(
        (256 * np.arange(C) + 227).astype(np.int32), (128, C)
    ).copy()
    iq = np.broadcast_to((256.0 * np.arange(C)).astype(np.float32), (128, C)).copy()
    ir = np.broadcast_to(np.arange(128).astype(np.float32), (128, 128)).copy()

    shared = {
        "dvec": dvec,
        "w_t": w_t,
        "bias_row": bias_row,
        "w2reg": w2reg,
        "ident": ident,
        "i2c227": i2c227,
        "iota_q": iq,
        "iota_r": ir,
    }
    if exact:
        shared["x_f32"] = np.ascontiguousarray(X).astype(np.float32)
    else:
        shared["x_bf"] = np.ascontiguousarray(X).astype(FDT)

    in_maps = []
    for c in range(n_cores):
        m = dict(shared)
        m["a_sh"] = A_bf[c * rows:(c + 1) * rows, :].view(FDT)
        in_maps.append(m)
    return in_maps


def kernel(D, X, A, W, b):
    from concourse.bass_utils import run_bass_kernel_spmd

    n, f_in = X.shape
    f_out = W.shape[0]
    rows = n // N_CORES
    nc = _get_nc(rows, n, f_in, f_out, EXACT)
    in_maps = host_inputs(D, X, A, W, b, N_CORES, EXACT)
    res = run_bass_kernel_spmd(nc, in_maps, core_ids=list(range(N_CORES)))
    out = np.concatenate([r["out_sh"] for r in res.results], axis=0)
    return out.astype(np.float32)



# revision 2
# speedup vs baseline: 1.8332x; 1.8332x over previous
"""GCN-style message passing kernel for Trainium2 (8 NeuronCores).

Math (see reference):
    deg    = diag(D)                      (== row sums of A by construction)
    j0(i)  = argmax_j (A[i,j] > 0)        (first neighbor; self-loops ensure >=1)
    coeff  = A * outer(1/sqrt(deg[j0]), 1/sqrt(deg))
    out    = leaky_relu((coeff @ X) @ W.T + b, 0.01)

Decomposition per core (rows sharded, 1024 rows/core):
    agg   = diag(r0) @ A_sh @ (diag(r) @ X)       r = 1/sqrt(deg), r0 = 1/sqrt(deg[j0])
    out   = leaky_relu(agg @ W.T + b)

A is 0/1 so it is exact in bf16. The host pre-transposes each core's row
slab of A (pure layout: materializes the column-major shard) so the device
loads A^T with large linear DMA descriptors instead of a 256B-packet
DMA-transpose. The big product A_sh @ Xs runs on the TensorEngine with A^T
tiles as the stationary operand. deg[j0] is recovered on-device:
  - a side matmul with a "position" matrix W2 (w[p] = 2^(100-p), one
    column per 128-node chunk) produces s[i,c] whose f32 EXPONENT encodes
    the first neighbor's offset within chunk c,
  - bit tricks + a free-dim min-reduce give first_j = 128*c* + jl*,
  - deg[first_j] is then gathered with a tiny bilinear form:
    onehot(c*)^T @ Dmat dotted with onehot(jl*), Dmat[q,r] = deg[128q+r].
"""

import numpy as np
import ml_dtypes

BF16 = ml_dtypes.bfloat16

N_NODES = 8192
F_IN = 256
F_OUT = 256
N_CORES = 8
ROWS = N_NODES // N_CORES  # rows per core

# accuracy mode: 'bf16' = single bf16 pass (~3e-3 rel err, tolerance 2e-2),
# 'exact' = f32 X split into two bf16 passes (err ~1e-5).
EXACT = 'bf16'

_BUILT = {}


def _build_nc(rows, n_nodes, f_in, f_out, mode, debug=False):
    exact = (mode == 'exact') or (mode is True)
    import concourse.bass as bass
    import concourse.tile as tile
    from concourse import bacc, mybir

    f32 = mybir.dt.float32
    bf = mybir.dt.bfloat16
    i32 = mybir.dt.int32
    Alu = mybir.AluOpType

    n_jblk = n_nodes // 128     # contraction blocks
    n_iblk = rows // 128        # output row blocks per core
    C = n_nodes // 128          # 128-node chunks (s columns) == n_jblk
    NB = n_jblk
    assert C <= 128 and n_nodes % 128 == 0 and rows % 128 == 0
    assert f_in % 128 == 0 and f_out <= 512

    nc = bacc.Bacc("TRN2", target_bir_lowering=False, debug=False)
    at_sh = nc.dram_tensor("at_sh", [n_nodes, rows], bf, kind="ExternalInput")
    dvec = nc.dram_tensor("dvec", [n_nodes], f32, kind="ExternalInput")
    if exact:
        x_in = nc.dram_tensor("x_f32", [n_nodes, f_in], f32, kind="ExternalInput")
    else:
        x_in = nc.dram_tensor("x_bf", [n_nodes, f_in], bf, kind="ExternalInput")
    w_t = nc.dram_tensor("w_t", [f_in, f_out], f32, kind="ExternalInput")
    bias_row = nc.dram_tensor("bias_row", [128, f_out], f32, kind="ExternalInput")
    w2reg_d = nc.dram_tensor("w2reg", [128, n_jblk, C], bf, kind="ExternalInput")
    ident_d = nc.dram_tensor("ident", [128, 128], bf, kind="ExternalInput")
    i2c227_d = nc.dram_tensor("i2c227", [128, C], i32, kind="ExternalInput")
    iq_d = nc.dram_tensor("iota_q", [128, C], f32, kind="ExternalInput")
    ir_d = nc.dram_tensor("iota_r", [128, 128], f32, kind="ExternalInput")
    out_d = nc.dram_tensor("out_sh", [rows, f_out], f32, kind="ExternalOutput")

    nfi = f_in // 128  # fi blocks for second matmul

    with tile.TileContext(nc) as tc:
        with (
            tc.tile_pool(name="singles", bufs=1) as singles,
            tc.tile_pool(name="apool", bufs=8) as apool,
            tc.tile_pool(name="xpool", bufs=3) as xpool,
            tc.tile_pool(name="work", bufs=2) as work,
            tc.tile_pool(name="pspool", bufs=8, space="PSUM") as pspool,
        ):
            # ---- constants / prep ----
            ident = singles.tile([128, 128], bf)
            nc.gpsimd.dma_start(ident[:], ident_d[:])
            i2c227 = singles.tile([128, C], i32)
            nc.gpsimd.dma_start(i2c227[:], i2c227_d[:])
            iq = singles.tile([128, C], f32)
            nc.gpsimd.dma_start(iq[:], iq_d[:])
            ir = singles.tile([128, 128], f32)
            nc.gpsimd.dma_start(ir[:], ir_d[:])
            bias_t = singles.tile([128, f_out], f32)
            nc.gpsimd.dma_start(bias_t[:], bias_row[:])

            # degrees: r = 1/sqrt(deg) laid out [p, nb]; Dmat[q, r] = deg[128q+r]
            dvec_t = singles.tile([128, NB], f32)
            nc.gpsimd.dma_start(dvec_t[:], dvec[:].rearrange("(nb p) -> p nb", p=128))
            dmat_f = singles.tile([C, 128], f32)
            nc.gpsimd.dma_start(dmat_f[:], dvec[:].rearrange("(q r) -> q r", r=128))
            dmat_b = singles.tile([C, 128], bf)
            nc.vector.tensor_copy(dmat_b[:], dmat_f[:])

            sq_t = singles.tile([128, NB], f32)
            nc.scalar.sqrt(sq_t[:], dvec_t[:])
            r_t = singles.tile([128, NB], f32)
            nc.vector.reciprocal(r_t[:], sq_t[:])

            # W^T in bf16 hi/lo: wthi/wtlo [128, nfi, f_out]
            wt_f = singles.tile([128, nfi, f_out], f32)
            nc.gpsimd.dma_start(
                wt_f[:], w_t[:].rearrange("(nf p) fo -> p nf fo", p=128)
            )
            wthi = singles.tile([128, nfi, f_out], bf)
            nc.vector.tensor_copy(wthi[:], wt_f[:])
            wtlo = singles.tile([128, nfi, f_out], bf)
            nc.vector.tensor_sub(wtlo[:], wt_f[:], wthi[:])

            # ---- moving operand: [Xs | W2] per j-block (bf16), + lo if exact
            xsw = singles.tile([128, n_jblk, f_in + C], bf)
            for jb in range(n_jblk):
                nc.gpsimd.dma_start(
                    xsw[:, jb, f_in:f_in + C], w2reg_d[:, jb, :]
                )
            if exact:
                xs_lo = singles.tile([128, n_jblk, f_in], bf)
            for jb in range(n_jblk):
                if exact:
                    xst = xpool.tile([128, f_in], f32, tag="xst")
                    nc.sync.dma_start(xst[:], x_in[jb * 128:(jb + 1) * 128, :])
                    xsf = xpool.tile([128, f_in], f32, tag="xsf")
                    nc.vector.tensor_scalar_mul(xsf[:], xst[:], r_t[:, jb:jb + 1])
                    nc.vector.tensor_copy(xsw[:, jb, 0:f_in], xsf[:])
                    nc.vector.tensor_sub(xs_lo[:, jb, :], xsf[:], xsw[:, jb, 0:f_in])
                else:
                    xst = xpool.tile([128, f_in], bf, tag="xst")
                    nc.gpsimd.dma_start(xst[:], x_in[jb * 128:(jb + 1) * 128, :])
                    nc.vector.tensor_scalar_mul(
                        xsw[:, jb, 0:f_in], xst[:], r_t[:, jb:jb + 1]
                    )

            # ---- main accumulation: agg = A_sh @ Xs ; s = A_sh @ W2
            ps_main = [
                pspool.tile([128, f_in + C], f32, tag="ps", name=f"ps_main{i}")
                for i in range(n_iblk)
            ]
            for jb in range(n_jblk):
                aslab = apool.tile([128, rows], bf, tag="aslab")
                nc.sync.dma_start(aslab[:], at_sh[jb * 128:(jb + 1) * 128, :])
                for ib in range(n_iblk):
                    lhsT = aslab[:, ib * 128:(ib + 1) * 128]
                    nc.tensor.matmul(
                        ps_main[ib][:, 0:f_in + C],
                        lhsT,
                        xsw[:, jb, :],
                        start=(jb == 0),
                        stop=(jb == n_jblk - 1) and not exact,
                    )
                    if exact:
                        nc.tensor.matmul(
                            ps_main[ib][:, 0:f_in],
                            lhsT,
                            xs_lo[:, jb, :],
                            start=False,
                            stop=(jb == n_jblk - 1),
                        )

            # ---- per row-block epilogue ----
            for ib in range(n_iblk):
                # drain psum: s and unscaled agg -> SBUF (releases the bank)
                s_sb = work.tile([128, C], f32, tag="s_sb")
                nc.scalar.copy(s_sb[:], ps_main[ib][:, f_in:f_in + C])
                agg_raw = work.tile([128, f_in], f32, tag="agg_raw")
                nc.scalar.copy(agg_raw[:], ps_main[ib][:, 0:f_in])
                e_u = work.tile([128, C], i32, tag="e_u")
                nc.vector.tensor_scalar(
                    e_u[:], s_sb[:].bitcast(i32), 23, None,
                    op0=Alu.logical_shift_right,
                )
                key = work.tile([128, C], i32, tag="key")
                nc.vector.scalar_tensor_tensor(
                    key[:], e_u[:], -1, i2c227[:], op0=Alu.mult, op1=Alu.add
                )
                msk = work.tile([128, C], i32, tag="msk")
                nc.vector.tensor_scalar(
                    msk[:], e_u[:], 0, 1 << 20, op0=Alu.is_equal, op1=Alu.mult
                )
                key2 = work.tile([128, C], i32, tag="key2")
                nc.vector.tensor_tensor(key2[:], key[:], msk[:], Alu.add)
                kmin = work.tile([128, 1], i32, tag="kmin")
                nc.vector.tensor_reduce(
                    kmin[:], key2[:], axis=mybir.AxisListType.X, op=Alu.min
                )
                # kmin = 256*c + jl  (c = chunk, jl = offset in chunk)
                jl2_i = work.tile([128, 1], i32, tag="jl2_i")
                nc.vector.tensor_scalar(
                    jl2_i[:], kmin[:], 127, None, op0=Alu.bitwise_and
                )
                c128_i = work.tile([128, 1], i32, tag="c128_i")
                nc.vector.tensor_scalar(
                    c128_i[:], kmin[:], -256, None, op0=Alu.bitwise_and
                )
                jl2_f = work.tile([128, 1], f32, tag="jl2_f")
                nc.vector.tensor_copy(jl2_f[:], jl2_i[:])
                c128_f = work.tile([128, 1], f32, tag="c128_f")
                nc.vector.tensor_copy(c128_f[:], c128_i[:])

                # onehots; gather deg[first_j] via oq^T @ Dmat then dot with or
                oq = work.tile([128, C], bf, tag="oq")
                nc.vector.tensor_scalar(
                    oq[:], iq[:], c128_f[:], None, op0=Alu.is_equal
                )
                orf = work.tile([128, 128], f32, tag="orf")
                nc.vector.tensor_scalar(
                    orf[:], ir[:], jl2_f[:], None, op0=Alu.is_equal
                )
                p_oqT = pspool.tile([C, 128], bf, tag="ps")
                nc.tensor.transpose(p_oqT[:], oq[:], ident[:])
                oqT = work.tile([C, 128], bf, tag="oqT")
                nc.scalar.copy(oqT[:], p_oqT[:])
                t1 = pspool.tile([128, 128], f32, tag="ps")
                nc.tensor.matmul(t1[:], oqT[:], dmat_b[:], start=True, stop=True)
                t1s = work.tile([128, 128], f32, tag="t1s")
                nc.scalar.copy(t1s[:], t1[:])
                ttr_scr = work.tile([128, 128], f32, tag="ttr_scr")
                nc.vector.tensor_tensor(ttr_scr[:], t1s[:], orf[:], Alu.mult)
                dj0 = work.tile([128, 1], f32, tag="dj0")
                nc.vector.reduce_sum(
                    dj0[:], ttr_scr[:], axis=mybir.AxisListType.X
                )
                sq0 = work.tile([128, 1], f32, tag="sq0")
                nc.scalar.sqrt(sq0[:], dj0[:])
                r0 = work.tile([128, 1], f32, tag="r0")
                nc.vector.reciprocal(r0[:], sq0[:])

                # agg scaled by r0, cast, transpose for the W matmul
                if exact:
                    agg_f = work.tile([128, f_in], f32, tag="agg_f")
                    nc.vector.tensor_scalar_mul(agg_f[:], agg_raw[:], r0[:])
                    agg_b = work.tile([128, f_in], bf, tag="agg_b")
                    nc.vector.tensor_copy(agg_b[:], agg_f[:])
                    agg_l = work.tile([128, f_in], bf, tag="agg_l")
                    nc.vector.tensor_sub(agg_l[:], agg_f[:], agg_b[:])
                else:
                    agg_b = work.tile([128, f_in], bf, tag="agg_b")
                    nc.vector.tensor_scalar_mul(agg_b[:], agg_raw[:], r0[:])

                aggTs = []
                for h in range(nfi):
                    p_aT = pspool.tile([128, 128], bf, tag="ps")
                    nc.tensor.transpose(
                        p_aT[:], agg_b[:, h * 128:(h + 1) * 128], ident[:]
                    )
                    aT = work.tile([128, 128], bf, tag=f"aT{h}")
                    nc.scalar.copy(aT[:], p_aT[:])
                    aggTs.append(aT)
                if exact:
                    aggTls = []
                    for h in range(nfi):
                        p_aT = pspool.tile([128, 128], bf, tag="ps")
                        nc.tensor.transpose(
                            p_aT[:], agg_l[:, h * 128:(h + 1) * 128], ident[:]
                        )
                        aTl = work.tile([128, 128], bf, tag=f"aTl{h}")
                        nc.scalar.copy(aTl[:], p_aT[:])
                        aggTls.append(aTl)

                ps2 = pspool.tile([128, f_out], f32, tag="ps")
                prods = []
                for h in range(nfi):
                    prods.append((aggTs[h], wthi[:, h, :]))
                    prods.append((aggTs[h], wtlo[:, h, :]))
                    if exact:
                        prods.append((aggTls[h], wthi[:, h, :]))
                for pi, (lhs, rhs) in enumerate(prods):
                    nc.tensor.matmul(
                        ps2[:], lhs[:], rhs,
                        start=(pi == 0), stop=(pi == len(prods) - 1),
                    )

                z = work.tile([128, f_out], f32, tag="z")
                nc.vector.tensor_add(z[:], ps2[:], bias_t[:])
                out_t = work.tile([128, f_out], f32, tag="out_t")
                nc.vector.scalar_tensor_tensor(
                    out_t[:], z[:], 0.01, z[:], op0=Alu.mult, op1=Alu.max
                )
                nc.sync.dma_start(
                    out_d[ib * 128:(ib + 1) * 128, :], out_t[:]
                )

    nc.finalize()
    return nc


def _get_nc(rows, n_nodes, f_in, f_out, mode, debug=False):
    key = (rows, n_nodes, f_in, f_out, mode, debug)
    if key not in _BUILT:
        _BUILT[key] = _build_nc(*key)
    return _BUILT[key]


def host_inputs(D, X, A, W, b, n_cores=N_CORES, mode=EXACT):
    """Build per-core input maps (pure slicing / layout / dtype re-encoding)."""
    exact = (mode == 'exact') or (mode is True)
    n, f_in = X.shape
    f_out = W.shape[0]
    rows = n // n_cores
    C = n // 128

    # A is 0/1: cast to bf16 is exact. Transpose each core's row slab so the
    # device loads A^T with linear DMA (column-major shard materialization).
    A_bf = (np.ascontiguousarray(A).view(np.uint32) >> 16).astype(np.uint16)
    dvec = np.ascontiguousarray(np.diagonal(D)).astype(np.float32)
    w_t = np.ascontiguousarray(W.T).astype(np.float32)
    bias_row = np.broadcast_to(b.astype(np.float32), (128, f_out)).copy()

    n_jblk = n // 128
    p = np.arange(128)
    w2reg = np.zeros((128, n_jblk, C), dtype=BF16)
    vals = (2.0 ** (100.0 - p)).astype(BF16)
    for bb in range(n_jblk):
        w2reg[p, bb, bb] = vals

    ident = np.eye(128, dtype=BF16)
    i2c227 = np.broadcast_to(
        (256 * np.arange(C) + 227).astype(np.int32), (128, C)
    ).copy()
    iq = np.broadcast_to((256.0 * np.arange(C)).astype(np.float32), (128, C)).copy()
    ir = np.broadcast_to(np.arange(128).astype(np.float32), (128, 128)).copy()

    shared = {
        "dvec": dvec,
        "w_t": w_t,
        "bias_row": bias_row,
        "w2reg": w2reg,
        "ident": ident,
        "i2c227": i2c227,
        "iota_q": iq,
        "iota_r": ir,
    }
    if exact:
        shared["x_f32"] = np.ascontiguousarray(X).astype(np.float32)
    else:
        shared["x_bf"] = np.ascontiguousarray(X).astype(BF16)

    in_maps = []
    for c in range(n_cores):
        m = dict(shared)
        m["at_sh"] = np.ascontiguousarray(
            A_bf[c * rows:(c + 1) * rows, :].T
        ).view(BF16)
        in_maps.append(m)
    return in_maps


def kernel(D, X, A, W, b):
    from concourse.bass_utils import run_bass_kernel_spmd

    n, f_in = X.shape
    f_out = W.shape[0]
    rows = n // N_CORES
    nc = _get_nc(rows, n, f_in, f_out, EXACT)
    in_maps = host_inputs(D, X, A, W, b, N_CORES, EXACT)
    res = run_bass_kernel_spmd(nc, in_maps, core_ids=list(range(N_CORES)))
    out = np.concatenate([r["out_sh"] for r in res.results], axis=0)
    return out.astype(np.float32)


# revision 13
# speedup vs baseline: 2.6917x; 1.4683x over previous
"""GCN-style message passing kernel for Trainium2 (8 NeuronCores).

Math (see reference):
    deg    = diag(D)                      (== row sums of A by construction)
    j0(i)  = argmax_j (A[i,j] > 0)        (first neighbor; self-loops ensure >=1)
    coeff  = A * outer(1/sqrt(deg[j0]), 1/sqrt(deg))
    out    = leaky_relu((coeff @ X) @ W.T + b, 0.01)

Decomposition per core (rows sharded, 1024 rows/core):
    agg   = diag(r0) @ A_sh @ (diag(r) @ X)       r = 1/sqrt(deg), r0 = 1/sqrt(deg[j0])
    out   = leaky_relu(agg @ W.T + b)

A is 0/1 so it is exact in bf16. The host materializes each core's shard of
A^T in a partition-major layout ([128 partitions, 64 jblk, 1024 rows]) so
the device pulls the whole thing with a few large linear DMA descriptors
(128KB contiguous per partition) instead of a 256B-packet DMA-transpose.
X is likewise shipped partition-major. The big product A_sh @ Xs runs on
the TensorEngine with A^T tiles as the stationary operand. deg[j0] is
recovered on-device via the exponent-encoding side matmul (W2) + bit
tricks + a tiny bilinear gather (see baseline docstring).
"""

import numpy as np
import ml_dtypes

BF16 = ml_dtypes.bfloat16

N_NODES = 8192
F_IN = 256
F_OUT = 256
N_CORES = 8
ROWS = N_NODES // N_CORES  # rows per core

_BUILT = {}

# A-group size: jblks per resident-A DMA chunk (16 DMAs of 4 jblks each)
AGRP = 4
# X-group size: jblks per X DMA chunk
XGRP = 8


def _build_nc(rows, n_nodes, f_in, f_out):
    import concourse.bass as bass
    import concourse.tile as tile
    from concourse import bacc, mybir

    f32 = mybir.dt.float32
    bf = mybir.dt.bfloat16
    i32 = mybir.dt.int32
    Alu = mybir.AluOpType

    n_jblk = n_nodes // 128     # contraction blocks
    n_iblk = rows // 128        # output row blocks per core
    C = n_nodes // 128          # 128-node chunks (s columns) == n_jblk
    NB = n_jblk
    n_ag = n_jblk // AGRP
    n_xg = n_jblk // XGRP
    assert C <= 128 and n_nodes % 128 == 0 and rows % 128 == 0
    assert f_in % 128 == 0 and f_out <= 512

    nc = bacc.Bacc("TRN2", target_bir_lowering=False, debug=False)
    at_sh = nc.dram_tensor("at_sh", [128, n_jblk, rows], bf, kind="ExternalInput")
    dvec = nc.dram_tensor("dvec", [n_nodes], f32, kind="ExternalInput")
    x_in = nc.dram_tensor("x_bf", [128, n_jblk, f_in], bf, kind="ExternalInput")
    w_t = nc.dram_tensor("w_t", [f_in, f_out], f32, kind="ExternalInput")
    bias_row = nc.dram_tensor("bias_row", [128, f_out], f32, kind="ExternalInput")
    w2vb_d = nc.dram_tensor("w2vb", [128, C], bf, kind="ExternalInput")
    ident_d = nc.dram_tensor("ident", [128, 128], bf, kind="ExternalInput")
    i2c227_d = nc.dram_tensor("i2c227", [128, C], i32, kind="ExternalInput")
    iq_d = nc.dram_tensor("iota_q", [128, C], f32, kind="ExternalInput")
    ir_d = nc.dram_tensor("iota_r", [128, 128], f32, kind="ExternalInput")
    out_d = nc.dram_tensor("out_sh", [rows, f_out], f32, kind="ExternalOutput")

    nfi = f_in // 128  # fi blocks for second matmul

    with tile.TileContext(nc) as tc:
        with (
            tc.tile_pool(name="singles", bufs=1) as singles,
            tc.tile_pool(name="xp", bufs=3) as xp,
            tc.tile_pool(name="work", bufs=2) as work,
            tc.tile_pool(name="pspool", bufs=8, space="PSUM") as pspool,
        ):
            # ---- degree vector first (gates the Xs scaling) ----
            dvec_t = singles.tile([128, NB], f32)
            nc.sync.dma_start(dvec_t[:], dvec[:].rearrange("(nb p) -> p nb", p=128))
            sq_t = singles.tile([128, NB], f32)
            nc.scalar.sqrt(sq_t[:], dvec_t[:])
            r_t = singles.tile([128, NB], f32)
            nc.vector.reciprocal(r_t[:], sq_t[:])

            # ---- A^T: 16 big linear loads on the sync HWDGE queue ----
            at_g = [singles.tile([128, AGRP, rows], bf, name=f"at_g{g}") for g in range(n_ag)]
            for g in range(n_ag):
                nc.sync.dma_start(
                    at_g[g][:], at_sh[:, g * AGRP:(g + 1) * AGRP, :]
                )

            # ---- constants (small, software DGE queue is fine) ----
            ident = singles.tile([128, 128], bf)
            nc.gpsimd.dma_start(ident[:], ident_d[:])
            i2c227 = singles.tile([128, C], i32)
            nc.gpsimd.dma_start(i2c227[:], i2c227_d[:])
            iq = singles.tile([128, C], f32)
            nc.gpsimd.dma_start(iq[:], iq_d[:])
            ir = singles.tile([128, 128], f32)
            nc.gpsimd.dma_start(ir[:], ir_d[:])
            bias_t = singles.tile([128, f_out], f32)
            nc.gpsimd.dma_start(bias_t[:], bias_row[:])
            w2vb = singles.tile([128, C], bf)
            nc.gpsimd.dma_start(w2vb[:], w2vb_d[:])
            dmat_f = singles.tile([C, 128], f32)
            nc.gpsimd.dma_start(dmat_f[:], dvec[:].rearrange("(q r) -> q r", r=128))
            dmat_b = singles.tile([C, 128], bf)
            nc.vector.tensor_copy(dmat_b[:], dmat_f[:])

            # W^T in bf16 hi/lo: wthi/wtlo [128, nfi, f_out]
            wt_f = singles.tile([128, nfi, f_out], f32)
            nc.gpsimd.dma_start(
                wt_f[:], w_t[:].rearrange("(nf p) fo -> p nf fo", p=128)
            )
            wthi = singles.tile([128, nfi, f_out], bf)
            nc.vector.tensor_copy(wthi[:], wt_f[:])
            wtlo = singles.tile([128, nfi, f_out], bf)
            nc.vector.tensor_sub(wtlo[:], wt_f[:], wthi[:])

            # ---- moving operand per j-block: [Xs | W2] (separate tiles) ----
            # X loaded partition-major in groups on the scalar HWDGE queue;
            # W2 diag block built on-device: (iq == 256*jb) * vals[p].
            xsw = []
            for g in range(n_xg):
                xr = xp.tile([128, XGRP, f_in], bf, tag="xr")
                nc.scalar.dma_start(xr[:], x_in[:, g * XGRP:(g + 1) * XGRP, :])
                for jl in range(XGRP):
                    jb = g * XGRP + jl
                    t = singles.tile([128, f_in + C], bf, name=f"xsw{jb}")
                    nc.vector.tensor_scalar_mul(
                        t[:, 0:f_in], xr[:, jl, :], r_t[:, jb:jb + 1]
                    )
                    nc.vector.scalar_tensor_tensor(
                        t[:, f_in:f_in + C], iq[:], 256.0 * jb, w2vb[:],
                        op0=Alu.is_equal, op1=Alu.mult,
                    )
                    xsw.append(t)

            # ---- main accumulation: agg = A_sh @ Xs ; s = A_sh @ W2 ----
            ps_main = [
                pspool.tile([128, f_in + C], f32, tag="ps", name=f"ps_main{i}")
                for i in range(n_iblk)
            ]
            for jb in range(n_jblk):
                asl = at_g[jb // AGRP]
                for ib in range(n_iblk):
                    lhsT = asl[:, jb % AGRP, ib * 128:(ib + 1) * 128]
                    nc.tensor.matmul(
                        ps_main[ib][:, 0:f_in + C],
                        lhsT,
                        xsw[jb][:],
                        start=(jb == 0),
                        stop=(jb == n_jblk - 1),
                    )

            # ---- per row-block epilogue ----
            for ib in range(n_iblk):
                # drain psum: s and unscaled agg -> SBUF (releases the bank)
                s_sb = work.tile([128, C], f32, tag="s_sb")
                nc.scalar.copy(s_sb[:], ps_main[ib][:, f_in:f_in + C])
                agg_raw = work.tile([128, f_in], f32, tag="agg_raw")
                nc.scalar.copy(agg_raw[:], ps_main[ib][:, 0:f_in])
                e_u = work.tile([128, C], i32, tag="e_u")
                nc.vector.tensor_scalar(
                    e_u[:], s_sb[:].bitcast(i32), 23, None,
                    op0=Alu.logical_shift_right,
                )
                key = work.tile([128, C], i32, tag="key")
                nc.vector.scalar_tensor_tensor(
                    key[:], e_u[:], -1, i2c227[:], op0=Alu.mult, op1=Alu.add
                )
                msk = work.tile([128, C], i32, tag="msk")
                nc.vector.tensor_scalar(
                    msk[:], e_u[:], 0, 1 << 20, op0=Alu.is_equal, op1=Alu.mult
                )
                key2 = work.tile([128, C], i32, tag="key2")
                nc.vector.tensor_tensor(key2[:], key[:], msk[:], Alu.add)
                kmin = work.tile([128, 1], i32, tag="kmin")
                nc.vector.tensor_reduce(
                    kmin[:], key2[:], axis=mybir.AxisListType.X, op=Alu.min
                )
                # kmin = 256*c + jl  (c = chunk, jl = offset in chunk)
                jl2_i = work.tile([128, 1], i32, tag="jl2_i")
                nc.vector.tensor_scalar(
                    jl2_i[:], kmin[:], 127, None, op0=Alu.bitwise_and
                )
                c128_i = work.tile([128, 1], i32, tag="c128_i")
                nc.vector.tensor_scalar(
                    c128_i[:], kmin[:], -256, None, op0=Alu.bitwise_and
                )
                jl2_f = work.tile([128, 1], f32, tag="jl2_f")
                nc.vector.tensor_copy(jl2_f[:], jl2_i[:])
                c128_f = work.tile([128, 1], f32, tag="c128_f")
                nc.vector.tensor_copy(c128_f[:], c128_i[:])

                # onehots; gather deg[first_j] via oq^T @ Dmat then dot with or
                oq = work.tile([128, C], bf, tag="oq")
                nc.vector.tensor_scalar(
                    oq[:], iq[:], c128_f[:], None, op0=Alu.is_equal
                )
                orf = work.tile([128, 128], f32, tag="orf")
                nc.vector.tensor_scalar(
                    orf[:], ir[:], jl2_f[:], None, op0=Alu.is_equal
                )
                p_oqT = pspool.tile([C, 128], bf, tag="ps")
                nc.tensor.transpose(p_oqT[:], oq[:], ident[:])
                oqT = work.tile([C, 128], bf, tag="oqT")
                nc.scalar.copy(oqT[:], p_oqT[:])
                t1 = pspool.tile([128, 128], f32, tag="ps")
                nc.tensor.matmul(t1[:], oqT[:], dmat_b[:], start=True, stop=True)
                t1s = work.tile([128, 128], f32, tag="t1s")
                nc.scalar.copy(t1s[:], t1[:])
                ttr_scr = work.tile([128, 128], f32, tag="ttr_scr")
                nc.vector.tensor_tensor(ttr_scr[:], t1s[:], orf[:], Alu.mult)
                dj0 = work.tile([128, 1], f32, tag="dj0")
                nc.vector.reduce_sum(
                    dj0[:], ttr_scr[:], axis=mybir.AxisListType.X
                )
                sq0 = work.tile([128, 1], f32, tag="sq0")
                nc.scalar.sqrt(sq0[:], dj0[:])
                r0 = work.tile([128, 1], f32, tag="r0")
                nc.vector.reciprocal(r0[:], sq0[:])

                # agg scaled by r0, cast, transpose for the W matmul
                agg_b = work.tile([128, f_in], bf, tag="agg_b")
                nc.vector.tensor_scalar_mul(agg_b[:], agg_raw[:], r0[:])

                aggTs = []
                for h in range(nfi):
                    p_aT = pspool.tile([128, 128], bf, tag="ps")
                    nc.tensor.transpose(
                        p_aT[:], agg_b[:, h * 128:(h + 1) * 128], ident[:]
                    )
                    aT = work.tile([128, 128], bf, tag=f"aT{h}")
                    nc.scalar.copy(aT[:], p_aT[:])
                    aggTs.append(aT)

                ps2 = pspool.tile([128, f_out], f32, tag="ps")
                prods = []
                for h in range(nfi):
                    prods.append((aggTs[h], wthi[:, h, :]))
                    prods.append((aggTs[h], wtlo[:, h, :]))
                for pi, (lhs, rhs) in enumerate(prods):
                    nc.tensor.matmul(
                        ps2[:], lhs[:], rhs,
                        start=(pi == 0), stop=(pi == len(prods) - 1),
                    )

                z = work.tile([128, f_out], f32, tag="z")
                nc.vector.tensor_add(z[:], ps2[:], bias_t[:])
                out_t = work.tile([128, f_out], f32, tag="out_t")
                nc.vector.scalar_tensor_tensor(
                    out_t[:], z[:], 0.01, z[:], op0=Alu.mult, op1=Alu.max
                )
                nc.sync.dma_start(
                    out_d[ib * 128:(ib + 1) * 128, :], out_t[:]
                )

    nc.finalize()
    return nc


def _get_nc(rows, n_nodes, f_in, f_out):
    key = (rows, n_nodes, f_in, f_out)
    if key not in _BUILT:
        _BUILT[key] = _build_nc(*key)
    return _BUILT[key]


def host_inputs(D, X, A, W, b, n_cores=N_CORES):
    """Build per-core input maps (pure slicing / layout / dtype re-encoding)."""
    n, f_in = X.shape
    f_out = W.shape[0]
    rows = n // n_cores
    C = n // 128
    n_jblk = n // 128

    # A is 0/1: cast to bf16 is exact. Materialize each core's column-major
    # (transposed) shard in partition-major layout [128, n_jblk, rows].
    A_bf = (np.ascontiguousarray(A).view(np.uint32) >> 16).astype(np.uint16)
    dvec = np.ascontiguousarray(np.diagonal(D)).astype(np.float32)
    w_t = np.ascontiguousarray(W.T).astype(np.float32)
    bias_row = np.broadcast_to(b.astype(np.float32), (128, f_out)).copy()

    p = np.arange(128)
    vals = (2.0 ** (100.0 - p)).astype(BF16)
    w2vb = np.broadcast_to(vals[:, None], (128, C)).copy()

    ident = np.eye(128, dtype=BF16)
    i2c227 = np.broadcast_to(
        (256 * np.arange(C) + 227).astype(np.int32), (128, C)
    ).copy()
    iq = np.broadcast_to((256.0 * np.arange(C)).astype(np.float32), (128, C)).copy()
    ir = np.broadcast_to(np.arange(128).astype(np.float32), (128, 128)).copy()

    # X partition-major: [p, jb, f] with node j = 128*jb + p
    x_pm = np.ascontiguousarray(
        X.astype(BF16).reshape(n_jblk, 128, f_in).transpose(1, 0, 2)
    )

    shared = {
        "dvec": dvec,
        "w_t": w_t,
        "bias_row": bias_row,
        "w2vb": w2vb,
        "ident": ident,
        "i2c227": i2c227,
        "iota_q": iq,
        "iota_r": ir,
        "x_bf": x_pm,
    }

    in_maps = []
    for c in range(n_cores):
        m = dict(shared)
        # A^T shard [n, rows] -> partition-major [p, jb, rows], j = 128*jb + p
        at = A_bf[c * rows:(c + 1) * rows, :].T  # [n, rows]
        m["at_sh"] = np.ascontiguousarray(
            at.reshape(n_jblk, 128, rows).transpose(1, 0, 2)
        ).view(BF16)
        in_maps.append(m)
    return in_maps


def kernel(D, X, A, W, b):
    from concourse.bass_utils import run_bass_kernel_spmd

    n, f_in = X.shape
    f_out = W.shape[0]
    rows = n // N_CORES
    nc = _get_nc(rows, n, f_in, f_out)
    in_maps = host_inputs(D, X, A, W, b, N_CORES)
    res = run_bass_kernel_spmd(nc, in_maps, core_ids=list(range(N_CORES)))
    out = np.concatenate([r["out_sh"] for r in res.results], axis=0)
    return out.astype(np.float32)


# revision 18
# speedup vs baseline: 2.8460x; 1.0573x over previous
"""GCN-style message passing kernel for Trainium2 (8 NeuronCores).

Math (see reference):
    deg    = diag(D)                      (== row sums of A by construction)
    j0(i)  = argmax_j (A[i,j] > 0)        (first neighbor; self-loops ensure >=1)
    coeff  = A * outer(1/sqrt(deg[j0]), 1/sqrt(deg))
    out    = leaky_relu((coeff @ X) @ W.T + b, 0.01)

Decomposition per core (rows sharded, 1024 rows/core):
    aggU  = A_sh @ (diag(r) @ X)          r = 1/sqrt(deg)
    out   = leaky_relu(r0 * (aggU @ W.T) + b),   r0 = 1/sqrt(deg[j0])
          = Lrelu_act(aggU @ W.T + sqrt(deg[j0]) * b, scale=r0)

A is 0/1 so it is exact in bf16. The host materializes each core's shard of
A^T in a partition-major layout ([128 partitions, 64 jblk, 1024 rows]) so
the device pulls the whole thing with a few large linear DMA descriptors
(128KB contiguous per partition) instead of a 256B-packet DMA-transpose.
X is likewise shipped partition-major. The big product A_sh @ Xs runs on
the TensorEngine with A^T tiles as the stationary operand. deg[j0] is
recovered on-device:
  - 64 extra moving columns W2 (w2[p, c] = 2^(100-p) iff chunk(p)==c)
    ride along the main matmul; s[i,c]'s f32 EXPONENT encodes the first
    neighbor's offset within chunk c,
  - a batched bit-trick chain + free-dim min-reduce gives
    first_j = 128*c* + jl*,
  - deg[first_j] is gathered with a tiny bilinear form per row block:
    onehot(c*)^T @ Dmat dotted with onehot(jl*), Dmat[q,r] = deg[128q+r].
The r0 scaling and bias ride the output path: bias enters the final psum
via a 1-partition matmul sqrt(dj0)^T (x) b, and the ScalarEngine applies
Lrelu with per-partition scale=r0 while draining psum.
"""

import numpy as np
import ml_dtypes

BF16 = ml_dtypes.bfloat16

N_NODES = 8192
F_IN = 256
F_OUT = 256
N_CORES = 8
ROWS = N_NODES // N_CORES  # rows per core

_BUILT = {}

# A-group size: jblks per resident-A DMA chunk (16 DMAs of 4 jblks each)
AGRP = 4
# X-group size: jblks per X DMA chunk
XGRP = 4


def _build_nc(rows, n_nodes, f_in, f_out):
    import concourse.bass as bass
    import concourse.tile as tile
    from concourse import bacc, mybir

    f32 = mybir.dt.float32
    bf = mybir.dt.bfloat16
    i32 = mybir.dt.int32
    Alu = mybir.AluOpType
    Act = mybir.ActivationFunctionType

    n_jblk = n_nodes // 128     # contraction blocks
    n_iblk = rows // 128        # output row blocks per core
    C = n_nodes // 128          # 128-node chunks (s columns) == n_jblk
    NB = n_jblk
    n_ag = n_jblk // AGRP
    n_xg = n_jblk // XGRP
    assert C <= 128 and n_nodes % 128 == 0 and rows % 128 == 0
    assert f_in % 128 == 0 and f_out <= 512

    nc = bacc.Bacc("TRN2", target_bir_lowering=False, debug=False)
    at_sh = nc.dram_tensor("at_sh", [128, n_jblk, rows], bf, kind="ExternalInput")
    dvec = nc.dram_tensor("dvec", [n_nodes], f32, kind="ExternalInput")
    x_in = nc.dram_tensor("x_bf", [128, n_jblk, f_in], bf, kind="ExternalInput")
    w_t = nc.dram_tensor("w_t", [f_in, f_out], f32, kind="ExternalInput")
    bhi_d = nc.dram_tensor("bhi", [1, f_out], bf, kind="ExternalInput")
    w2vb_d = nc.dram_tensor("w2vb", [128, C], bf, kind="ExternalInput")
    ident_d = nc.dram_tensor("ident", [128, 128], bf, kind="ExternalInput")
    i2c227_d = nc.dram_tensor("i2c227", [128, n_iblk, C], i32, kind="ExternalInput")
    iq_d = nc.dram_tensor("iota_q", [128, C], f32, kind="ExternalInput")
    ir_d = nc.dram_tensor("iota_r", [128, 128], f32, kind="ExternalInput")
    out_d = nc.dram_tensor("out_sh", [rows, f_out], f32, kind="ExternalOutput")

    nfi = f_in // 128  # fi blocks for second matmul

    with tile.TileContext(nc) as tc:
        with (
            tc.tile_pool(name="singles", bufs=1) as singles,
            tc.tile_pool(name="xp", bufs=3) as xp,
            tc.tile_pool(name="work", bufs=2) as work,
            tc.tile_pool(name="pspool", bufs=8, space="PSUM") as pspool,
        ):
            # ---- gating constants first, on the two HWDGE queues ----
            dvec_t = singles.tile([128, NB], f32)
            nc.sync.dma_start(dvec_t[:], dvec[:].rearrange("(nb p) -> p nb", p=128))
            iq = singles.tile([128, C], f32)
            nc.scalar.dma_start(iq[:], iq_d[:])
            w2vb = singles.tile([128, C], bf)
            nc.scalar.dma_start(w2vb[:], w2vb_d[:])

            sq_t = singles.tile([128, NB], f32)
            nc.scalar.sqrt(sq_t[:], dvec_t[:])
            r_t = singles.tile([128, NB], f32)
            nc.vector.reciprocal(r_t[:], sq_t[:])

            # ---- A^T: 16 big linear loads on the sync HWDGE queue ----
            at_g = [singles.tile([128, AGRP, rows], bf, name=f"at_g{g}")
                    for g in range(n_ag)]
            for g in range(n_ag):
                nc.sync.dma_start(
                    at_g[g][:], at_sh[:, g * AGRP:(g + 1) * AGRP, :]
                )

            # ---- moving operand per j-block: [Xs | W2] (separate tiles) ----
            # X loaded partition-major in groups on the scalar HWDGE queue;
            # W2 diag block built on-device: (iq == 256*jb) * vals[p].
            xsw = []
            for g in range(n_xg):
                xr = xp.tile([128, XGRP, f_in], bf, tag="xr")
                nc.scalar.dma_start(xr[:], x_in[:, g * XGRP:(g + 1) * XGRP, :])
                for jl in range(XGRP):
                    jb = g * XGRP + jl
                    t = singles.tile([128, f_in + C], bf, name=f"xsw{jb}")
                    nc.vector.tensor_scalar_mul(
                        t[:, 0:f_in], xr[:, jl, :], r_t[:, jb:jb + 1]
                    )
                    nc.vector.scalar_tensor_tensor(
                        t[:, f_in:f_in + C], iq[:], 256.0 * jb, w2vb[:],
                        op0=Alu.is_equal, op1=Alu.mult,
                    )
                    xsw.append(t)

            # ---- remaining constants (scalar HWDGE, after X groups) ----
            wt_f = singles.tile([128, nfi, f_out], f32)
            nc.scalar.dma_start(
                wt_f[:], w_t[:].rearrange("(nf p) fo -> p nf fo", p=128)
            )
            wthi = singles.tile([128, nfi, f_out], bf)
            nc.vector.tensor_copy(wthi[:], wt_f[:])
            wtlo = singles.tile([128, nfi, f_out], bf)
            nc.vector.tensor_sub(wtlo[:], wt_f[:], wthi[:])
            ident = singles.tile([128, 128], bf)
            nc.scalar.dma_start(ident[:], ident_d[:])
            i2c227 = singles.tile([128, n_iblk, C], i32)
            nc.scalar.dma_start(i2c227[:], i2c227_d[:])
            ir = singles.tile([128, 128], f32)
            nc.scalar.dma_start(ir[:], ir_d[:])
            bhi = singles.tile([1, f_out], bf)
            nc.scalar.dma_start(bhi[:], bhi_d[:])
            dmat_f = singles.tile([C, 128], f32)
            nc.scalar.dma_start(dmat_f[:], dvec[:].rearrange("(q r) -> q r", r=128))
            dmat_b = singles.tile([C, 128], bf)
            nc.vector.tensor_copy(dmat_b[:], dmat_f[:])

            # ---- main accumulation: agg = A_sh @ Xs ; s = A_sh @ W2 ----
            ps_main = [
                pspool.tile([128, f_in + C], f32, tag="ps", name=f"ps_main{i}")
                for i in range(n_iblk)
            ]
            for jb in range(n_jblk):
                asl = at_g[jb // AGRP]
                for ib in range(n_iblk):
                    lhsT = asl[:, jb % AGRP, ib * 128:(ib + 1) * 128]
                    nc.tensor.matmul(
                        ps_main[ib][:, 0:f_in + C],
                        lhsT,
                        xsw[jb][:],
                        start=(jb == 0),
                        stop=(jb == n_jblk - 1),
                    )

            # ---- epilogue, stage-major across all row blocks ----
            # Stage 1: drain psum -> SBUF (s in f32; agg unscaled in bf16),
            # freeing all psum banks for the gather/W matmuls.
            s_all = singles.tile([128, n_iblk, C], f32)
            agg_bu = singles.tile([128, n_iblk, f_in], bf)
            for ib in range(n_iblk):
                nc.scalar.copy(s_all[:, ib, :], ps_main[ib][:, f_in:f_in + C])
            for ib in range(n_iblk):
                nc.scalar.activation(
                    agg_bu[:, ib, :], ps_main[ib][:, 0:f_in], Act.Copy
                )

            # Stage 2: batched first-neighbor decode on the whole [128, 8*64]
            e_u = singles.tile([128, n_iblk, C], i32)
            nc.vector.tensor_scalar(
                e_u[:], s_all[:].bitcast(i32), 23, None,
                op0=Alu.logical_shift_right,
            )
            key = singles.tile([128, n_iblk, C], i32)
            nc.vector.scalar_tensor_tensor(
                key[:], e_u[:], -1, i2c227[:], op0=Alu.mult, op1=Alu.add
            )
            # msk = (e_u==0)<<20 overwrites e_u (no longer needed), then
            # key2 = key + msk overwrites key
            nc.vector.tensor_scalar(
                e_u[:], e_u[:], 0, 1 << 20, op0=Alu.is_equal, op1=Alu.mult
            )
            nc.vector.tensor_tensor(key[:], key[:], e_u[:], Alu.add)
            kmin = singles.tile([128, n_iblk], i32)
            nc.vector.tensor_reduce(
                kmin[:], key[:], axis=mybir.AxisListType.X, op=Alu.min
            )
            # kmin = 256*c + jl  (c = chunk, jl = offset in chunk)
            jl2_i = singles.tile([128, n_iblk], i32)
            nc.vector.tensor_scalar(
                jl2_i[:], kmin[:], 127, None, op0=Alu.bitwise_and
            )
            c128_i = singles.tile([128, n_iblk], i32)
            nc.vector.tensor_scalar(
                c128_i[:], kmin[:], -256, None, op0=Alu.bitwise_and
            )
            jl2_f = singles.tile([128, n_iblk], f32)
            nc.vector.tensor_copy(jl2_f[:], jl2_i[:])
            c128_f = singles.tile([128, n_iblk], f32)
            nc.vector.tensor_copy(c128_f[:], c128_i[:])

            # Stage 3: per-block onehots (need [128,1] per-partition scalars)
            oq_all = singles.tile([128, n_iblk, C], bf)
            orf_all = singles.tile([128, n_iblk, 128], bf)
            for ib in range(n_iblk):
                nc.vector.tensor_scalar(
                    oq_all[:, ib, :], iq[:], c128_f[:, ib:ib + 1], None,
                    op0=Alu.is_equal,
                )
            for ib in range(n_iblk):
                nc.vector.tensor_scalar(
                    orf_all[:, ib, :], ir[:], jl2_f[:, ib:ib + 1], None,
                    op0=Alu.is_equal,
                )

            # Stage 4: gather deg[first_j]: t1 = oq^T @ Dmat per block
            t1_all = singles.tile([128, n_iblk, 128], bf)
            for ib in range(n_iblk):
                p_oqT = pspool.tile([C, 128], bf, tag="ps")
                nc.tensor.transpose(p_oqT[:], oq_all[:, ib, :], ident[:])
                oqT = work.tile([C, 128], bf, tag="oqT")
                nc.scalar.copy(oqT[:], p_oqT[:])
                t1 = pspool.tile([128, 128], f32, tag="ps")
                nc.tensor.matmul(t1[:], oqT[:], dmat_b[:], start=True, stop=True)
                nc.scalar.copy(t1_all[:, ib, :], t1[:])

            # Stage 5: dj0 = sum(t1 * onehot(jl)), then r0 terms (batched)
            ttr = singles.tile([128, n_iblk, 128], bf)
            nc.vector.tensor_tensor(ttr[:], t1_all[:], orf_all[:], Alu.mult)
            dj0 = singles.tile([128, n_iblk], f32)
            nc.vector.tensor_reduce(
                dj0[:], ttr[:], axis=mybir.AxisListType.X, op=Alu.add
            )
            sq0 = singles.tile([128, n_iblk], f32)
            nc.scalar.sqrt(sq0[:], dj0[:])
            r0 = singles.tile([128, n_iblk], f32)
            nc.vector.reciprocal(r0[:], sq0[:])
            # sqrt(dj0) columns transposed to [1, 128] rows (base partition 0)
            # for the per-block bias matmul
            sq0b = singles.tile([128, n_iblk], bf)
            nc.vector.tensor_copy(sq0b[:], sq0[:])
            sq0T = []
            for ib in range(n_iblk):
                p_s1 = pspool.tile([1, 128], bf, tag="ps")
                nc.tensor.transpose(p_s1[:], sq0b[:, ib:ib + 1], ident[:])
                s1 = singles.tile([1, 128], bf, name=f"sq0T{ib}")
                nc.scalar.copy(s1[:], p_s1[:])
                sq0T.append(s1)

            # Stage 6: per block: transpose aggU, W matmuls (+bias), Lrelu
            for ib in range(n_iblk):
                aggTs = []
                for h in range(nfi):
                    p_aT = pspool.tile([128, 128], bf, tag="ps")
                    nc.tensor.transpose(
                        p_aT[:], agg_bu[:, ib, h * 128:(h + 1) * 128], ident[:]
                    )
                    aT = work.tile([128, 128], bf, tag=f"aT{h}")
                    nc.scalar.copy(aT[:], p_aT[:])
                    aggTs.append(aT)

                ps2 = pspool.tile([128, f_out], f32, tag="ps")
                prods = []
                for h in range(nfi):
                    prods.append((aggTs[h], wthi[:, h, :]))
                    prods.append((aggTs[h], wtlo[:, h, :]))
                for pi, (lhs, rhs) in enumerate(prods):
                    nc.tensor.matmul(
                        ps2[:], lhs[:], rhs,
                        start=(pi == 0), stop=False,
                    )
                # bias: += sqrt(dj0)^T (x) b   (1-partition outer product)
                nc.tensor.matmul(
                    ps2[:], sq0T[ib][:], bhi[:],
                    start=False, stop=True,
                )
                out_t = work.tile([128, f_out], f32, tag="out_t")
                nc.scalar.activation(
                    out_t[:], ps2[:], Act.Lrelu,
                    scale=r0[:, ib:ib + 1], alpha=0.01,
                )
                nc.sync.dma_start(
                    out_d[ib * 128:(ib + 1) * 128, :], out_t[:]
                )

    nc.finalize()
    return nc


def _get_nc(rows, n_nodes, f_in, f_out):
    key = (rows, n_nodes, f_in, f_out)
    if key not in _BUILT:
        _BUILT[key] = _build_nc(*key)
    return _BUILT[key]


def host_inputs(D, X, A, W, b, n_cores=N_CORES):
    """Build per-core input maps (pure slicing / layout / dtype re-encoding)."""
    n, f_in = X.shape
    f_out = W.shape[0]
    rows = n // n_cores
    C = n // 128
    n_jblk = n // 128
    n_iblk = rows // 128

    # A is 0/1: cast to bf16 is exact. Materialize each core's column-major
    # (transposed) shard in partition-major layout [128, n_jblk, rows].
    A_bf = (np.ascontiguousarray(A).view(np.uint32) >> 16).astype(np.uint16)
    dvec = np.ascontiguousarray(np.diagonal(D)).astype(np.float32)
    w_t = np.ascontiguousarray(W.T).astype(np.float32)

    p = np.arange(128)
    vals = (2.0 ** (100.0 - p)).astype(BF16)
    w2vb = np.broadcast_to(vals[:, None], (128, C)).copy()

    ident = np.eye(128, dtype=BF16)
    i2c227 = np.broadcast_to(
        (256 * np.arange(C) + 227).astype(np.int32), (128, n_iblk, C)
    ).copy()
    iq = np.broadcast_to((256.0 * np.arange(C)).astype(np.float32), (128, C)).copy()
    ir = np.broadcast_to(np.arange(128).astype(np.float32), (128, 128)).copy()
    bhi = b.astype(BF16).reshape(1, f_out)

    # X partition-major: [p, jb, f] with node j = 128*jb + p
    x_pm = np.ascontiguousarray(
        X.astype(BF16).reshape(n_jblk, 128, f_in).transpose(1, 0, 2)
    )

    shared = {
        "dvec": dvec,
        "w_t": w_t,
        "bhi": bhi,
        "w2vb": w2vb,
        "ident": ident,
        "i2c227": i2c227,
        "iota_q": iq,
        "iota_r": ir,
        "x_bf": x_pm,
    }

    in_maps = []
    for c in range(n_cores):
        m = dict(shared)
        # A^T shard [n, rows] -> partition-major [p, jb, rows], j = 128*jb + p
        at = A_bf[c * rows:(c + 1) * rows, :].T  # [n, rows]
        m["at_sh"] = np.ascontiguousarray(
            at.reshape(n_jblk, 128, rows).transpose(1, 0, 2)
        ).view(BF16)
        in_maps.append(m)
    return in_maps


def kernel(D, X, A, W, b):
    from concourse.bass_utils import run_bass_kernel_spmd

    n, f_in = X.shape
    f_out = W.shape[0]
    rows = n // N_CORES
    nc = _get_nc(rows, n, f_in, f_out)
    in_maps = host_inputs(D, X, A, W, b, N_CORES)
    res = run_bass_kernel_spmd(nc, in_maps, core_ids=list(range(N_CORES)))
    out = np.concatenate([r["out_sh"] for r in res.results], axis=0)
    return out.astype(np.float32)
